# revision 1
# baseline (speedup 1.0000x reference)
"""Trainium2 Bass kernel: 3x3 conv (NCHW 32x256x56x56, 256->256ch, pad 1) with
a host-expanded synthesized weight, data-parallel over 8 NeuronCores.

Conv as implicit GEMM: for each of the 9 kernel taps, a matmul over a
zero-padded (58x58, padded on host) input image held in SBUF with input
channels on partitions, accumulating over 18 matmuls (9 taps x 2 channel
tiles) in PSUM.  fp16 operands (fp32 accumulate) keep the PE at 1 col/cycle
with LDWEIGHTS fully hidden via fast-weight-load; N = 8 rows x 56 cols = 448
per matmul (PSUM-bank limit is 512 fp32).  Input DMAs are band-split and
spread over both HWDGE rings (sync + scalar), and ~75 junk warmup matmuls
keep the HAM clock gate at 8/8 until the first real matmul (~12us in).
Measured: ~218-228us per core (HFU ~86%); PE matmul-stream floor is ~190us.
"""

import numpy as np

# Problem constants (hardcoded per contract; kernel.py must be self-contained)
OOC, OIC, K1, K2 = 64, 64, 3, 3
R0, R1 = 4, 4
N_CORES = 8
BATCH = 32
N_PER_CORE = BATCH // N_CORES  # 4
C = 256
H = W = 56
HP = WP = H + 2  # zero-padded spatial (padding applied on host)
RB = 8           # output rows per matmul chunk -> N = RB*W = 448
NCH = H // RB    # 7 chunks
KT = C // 128    # 2 input-channel tiles
MT = C // 128    # 2 output-channel tiles
POS = K1 * K2    # 9 kernel taps

# Input-image DMA bands (padded-row ranges): first band covers exactly
# chunk 0 so compute starts early; chunk b needs padded rows 8b .. 8b+9.
BANDS = [(0, 10), (10, 16), (26, 16), (42, 16)]

_NC_CACHE = {}
LAST_RESULT = {}  # test.py introspection: last BassKernelResults


def _expand_weight(weight, alphas, betas):
    """W[p0*64+i, p1*64+j, ky, kx] = w[i,j,ky,kx] * a[p0,p1] / (1+exp(w*b[p0,p1]))."""
    w = weight.astype(np.float32)[None, None]            # (1,1,64,64,3,3)
    a = alphas.astype(np.float32).reshape(R0, R1)[:, :, None, None, None, None]
    b = betas.astype(np.float32).reshape(R0, R1)[:, :, None, None, None, None]
    act = w * a / (1.0 + np.exp(w * b))                  # (4,4,64,64,3,3)
    return act.transpose(0, 2, 1, 3, 4, 5).reshape(R0 * OOC, R1 * OIC, K1, K2)


def _host_prep(x, weight, alphas, betas, bias):
    x = np.asarray(x, dtype=np.float32).astype(np.float16)
    xpad = np.pad(x, ((0, 0), (0, 0), (1, 1), (1, 1)))
    Wfull = _expand_weight(np.asarray(weight), np.asarray(alphas),
                           np.asarray(betas))            # (256,256,3,3)
    # lhsT layout: [ci_local(128 partitions), kt, mt, pos, co_local(128)]
    Wt = Wfull.transpose(1, 0, 2, 3).reshape(C, C, POS)  # (ci, co, pos)
    w_arr = np.ascontiguousarray(
        Wt.reshape(KT, 128, MT, 128, POS).transpose(1, 0, 2, 4, 3)
    ).astype(np.float16)
    b_arr = np.ascontiguousarray(
        np.asarray(bias, dtype=np.float32).reshape(MT, 128).T)
    return xpad, w_arr, b_arr


def _build_nc():
    import concourse.mybir as mybir
    import concourse.tile as tile
    from concourse import bacc

    fp32 = mybir.dt.float32
    fp16 = mybir.dt.float16

    nc = bacc.Bacc("TRN2", target_bir_lowering=False, debug=False,
                   num_devices=N_CORES)

    x_d = nc.dram_tensor("x", [N_PER_CORE, C, HP, WP], fp16,
                         kind="ExternalInput")
    w_d = nc.dram_tensor("w", [128, KT, MT, POS, 128], fp16,
                         kind="ExternalInput")
    b_d = nc.dram_tensor("b", [128, MT], fp32, kind="ExternalInput")
    o_d = nc.dram_tensor("out", [N_PER_CORE, C, H, W], fp32,
                         kind="ExternalOutput")

    # Two HWDGE rings: sync carries kt=0 traffic, scalar carries kt=1.
    def ring(kt):
        return nc.sync if kt == 0 else nc.scalar

    with tile.TileContext(nc) as tc:
        with (
            tc.tile_pool(name="const", bufs=1) as const_pool,
            tc.tile_pool(name="xpad", bufs=1) as xp_pool,
            tc.tile_pool(name="ot", bufs=4) as out_pool,
            tc.tile_pool(name="ps", bufs=6, space="PSUM") as psum_pool,
        ):
            w_sb = const_pool.tile([128, KT, MT, POS, 128], fp16,
                                   name="w_sb", tag="w_sb")
            b_sb = const_pool.tile([128, MT], fp32, name="b_sb", tag="b_sb")

            # PE warmup: ~3.4us of junk matmuls on scratch SBUF during the
            # initial DMA wait flips the HAM clock gate to 8/8 before the
            # real stream starts (and costs nothing - PE is idle anyway).
            warm_in = const_pool.tile([128, 128], fp16, name="warm_in",
                                      tag="warm_in")
            warm_ps = psum_pool.tile([128, 64], fp32, name="warm_ps",
                                     tag="warm_ps", bufs=1)
            nc.vector.memset(warm_in[:], 0.0)
            for _ in range(75):
                nc.tensor.matmul(warm_ps[:], warm_in[:], warm_in[:, 0:64])

            # Double-buffered padded input images (pad arrives from host).
            xp = [[xp_pool.tile([128, HP, WP], fp16, name=f"xp{par}_{kt}",
                                tag=f"xp{par}_{kt}")
                   for kt in range(KT)] for par in range(2)]

            xap = x_d.ap()
            oap = o_d.ap()

            def band_dma(n, par, r0, nr):
                for kt in range(KT):
                    ring(kt).dma_start(
                        xp[par][kt][:, r0:r0 + nr, :],
                        xap[n, kt * 128:(kt + 1) * 128, r0:r0 + nr, :])

            # Head ring order: image-0 band0 first (smallest first-MM
            # critical path), then mt0 weights with the first kernel taps
            # (pos 0-2) ahead so MM#1 waits for the fewest bytes, then the
            # rest.  All head DMAs race the PE warmup.
            band_dma(0, 0, *BANDS[0])
            for kt in range(KT):
                ring(kt).dma_start(w_sb[:, kt, 0, 0:3], w_d.ap()[:, kt, 0, 0:3])
            for kt in range(KT):
                ring(kt).dma_start(w_sb[:, kt, 0, 3:POS],
                                   w_d.ap()[:, kt, 0, 3:POS])
            for kt in range(KT):
                ring(kt).dma_start(w_sb[:, kt, 1], w_d.ap()[:, kt, 1])
            nc.scalar.dma_start(b_sb[:], b_d.ap())

            for n in range(N_PER_CORE):
                par = n % 2
                for r0, nr in (BANDS[1:] if n == 0 else BANDS):
                    band_dma(n, par, r0, nr)
                for ch in range(NCH):
                    y0 = ch * RB
                    for mt in range(MT):
                        ps = psum_pool.tile([128, RB, W], fp32,
                                            name="ps", tag="ps")
                        first = True
                        for kt in range(KT):
                            for dy in range(K1):
                                for dx in range(K2):
                                    pos = dy * K2 + dx
                                    last = (kt == KT - 1 and pos == POS - 1)
                                    nc.tensor.matmul(
                                        ps[:, :, :],
                                        w_sb[:, kt, mt, pos, :],
                                        xp[par][kt][:, y0 + dy:y0 + dy + RB,
                                                    dx:dx + W],
                                        start=first, stop=last,
                                    )
                                    first = False
                        ot = out_pool.tile([128, RB, W], fp32,
                                           name="ot", tag="ot")
                        nc.vector.tensor_scalar_add(ot[:], ps[:],
                                                    b_sb[:, mt:mt + 1])
                        ring(mt).dma_start(
                            oap[n, mt * 128:(mt + 1) * 128, y0:y0 + RB, :],
                            ot[:])
    nc.compile()
    return nc


def get_nc():
    if "nc" not in _NC_CACHE:
        _NC_CACHE["nc"] = _build_nc()
    return _NC_CACHE["nc"]


def kernel(x, weight, alphas, betas, bias):
    from concourse.bass_utils import run_bass_kernel_spmd

    xpad, w_arr, b_arr = _host_prep(x, weight, alphas, betas, bias)
    nc = get_nc()
    in_maps = [
        {"x": xpad[i * N_PER_CORE:(i + 1) * N_PER_CORE], "w": w_arr,
         "b": b_arr}
        for i in range(N_CORES)
    ]
    res = run_bass_kernel_spmd(nc, in_maps, core_ids=list(range(N_CORES)))
    LAST_RESULT["res"] = res
    return np.concatenate([r["out"] for r in res.results], axis=0)



# revision 3
# speedup vs baseline: 1.1492x; 1.1492x over previous
"""Trainium2 Bass kernel: 3x3 conv (NCHW 32x256x56x56, 256->256ch, pad 1) with
a host-expanded synthesized weight, data-parallel over 8 NeuronCores.

Conv as implicit GEMM over the FLAT zero-padded image: for each of the 9
kernel taps, a matmul streams a CONTIGUOUS 464-element window (8 rows x 58
padded cols) of the flattened 58x58 image -- row-boundary columns compute
garbage that lands in PSUM columns 56/57 of each row and is skipped on
drain.  Contiguous streaming avoids the per-row-segment PE stall of a
strided rhs (~40ns/matmul measured).  fp16 operands (fp32 accumulate),
18 accumulating matmuls per 8-row output chunk (9 taps x 2 channel tiles),
N = 464 <= 512 PSUM-bank fp32 limit.  Input DMAs band-split over the sync +
scalar HWDGE rings; ~20 junk warmup matmuls bridge the HAM clock-gate while
the first bands land; final output DMAs split across 4 rings to cut the
drain tail.
"""

import numpy as np

# Problem constants (hardcoded per contract; kernel.py must be self-contained)
OOC, OIC, K1, K2 = 64, 64, 3, 3
R0, R1 = 4, 4
N_CORES = 8
BATCH = 32
N_PER_CORE = BATCH // N_CORES  # 4
C = 256
H = W = 56
HP = WP = H + 2  # zero-padded spatial (padding applied on host)
FLAT = HP * WP   # 3364
XTAIL = 2        # window overrun past the image for the last chunk
RB = 8           # output rows per matmul chunk -> N = RB*WP = 464
NCH = H // RB    # 7 chunks
KT = C // 128    # 2 input-channel tiles
MT = C // 128    # 2 output-channel tiles
POS = K1 * K2    # 9 kernel taps
NWIN = RB * WP   # 464 matmul columns

# Input-image DMA bands (padded-row ranges): first band covers exactly
# chunk 0 so compute starts early; chunk b needs padded rows 8b .. 8b+9.
BANDS = [(0, 10), (10, 16), (26, 16), (42, 16)]

_NC_CACHE = {}
LAST_RESULT = {}  # test.py introspection: last BassKernelResults


def _expand_weight(weight, alphas, betas):
    """W[p0*64+i, p1*64+j, ky, kx] = w[i,j,ky,kx] * a[p0,p1] / (1+exp(w*b[p0,p1]))."""
    w = weight.astype(np.float32)[None, None]            # (1,1,64,64,3,3)
    a = alphas.astype(np.float32).reshape(R0, R1)[:, :, None, None, None, None]
    b = betas.astype(np.float32).reshape(R0, R1)[:, :, None, None, None, None]
    act = w * a / (1.0 + np.exp(w * b))                  # (4,4,64,64,3,3)
    return act.transpose(0, 2, 1, 3, 4, 5).reshape(R0 * OOC, R1 * OIC, K1, K2)


def _host_prep(x, weight, alphas, betas, bias):
    x = np.asarray(x, dtype=np.float32).astype(np.float16)
    xpad = np.pad(x, ((0, 0), (0, 0), (1, 1), (1, 1)))
    xpad = np.ascontiguousarray(xpad).reshape(BATCH, C, FLAT)
    Wfull = _expand_weight(np.asarray(weight), np.asarray(alphas),
                           np.asarray(betas))            # (256,256,3,3)
    # lhsT layout: [ci_local(128 partitions), kt, mt, pos, co_local(128)]
    Wt = Wfull.transpose(1, 0, 2, 3).reshape(C, C, POS)  # (ci, co, pos)
    w_arr = np.ascontiguousarray(
        Wt.reshape(KT, 128, MT, 128, POS).transpose(1, 0, 2, 4, 3)
    ).astype(np.float16)
    b_arr = np.ascontiguousarray(
        np.asarray(bias, dtype=np.float32).reshape(MT, 128).T)
    return xpad, w_arr, b_arr


def _build_nc():
    import concourse.mybir as mybir
    import concourse.tile as tile
    from concourse import bacc

    fp32 = mybir.dt.float32
    fp16 = mybir.dt.float16

    nc = bacc.Bacc("TRN2", target_bir_lowering=False, debug=False,
                   num_devices=N_CORES)

    x_d = nc.dram_tensor("x", [N_PER_CORE, C, FLAT], fp16,
                         kind="ExternalInput")
    w_d = nc.dram_tensor("w", [128, KT, MT, POS, 128], fp16,
                         kind="ExternalInput")
    b_d = nc.dram_tensor("b", [128, MT], fp32, kind="ExternalInput")
    o_d = nc.dram_tensor("out", [N_PER_CORE, C, H, W], fp32,
                         kind="ExternalOutput")

    # Two HWDGE rings carry the steady-state traffic: sync has kt=0 input
    # bands + mt=0 outputs, scalar has kt=1 + mt=1.
    def ring(kt):
        return nc.sync if kt == 0 else nc.scalar

    with tile.TileContext(nc) as tc:
        with (
            tc.tile_pool(name="sb", bufs=1) as sb_pool,
            tc.tile_pool(name="ps", bufs=6, space="PSUM") as psum_pool,
        ):
            w_sb = sb_pool.tile([128, KT, MT, POS, 128], fp16,
                                name="w_sb", tag="w_sb")
            b_sb = sb_pool.tile([128, MT], fp32, name="b_sb", tag="b_sb")

            # PE warmup: junk matmuls on scratch SBUF during the initial DMA
            # wait start the HAM busy window so the clock gate flips to 8/8
            # shortly after the real stream starts.
            warm_in = sb_pool.tile([128, 128], fp16, name="warm_in",
                                   tag="warm_in")
            warm_ps = psum_pool.tile([128, 64], fp32, name="warm_ps",
                                     tag="warm_ps", bufs=1)
            nc.vector.memset(warm_in[:], 0.0)
            for _ in range(20):
                nc.tensor.matmul(warm_ps[:], warm_in[:], warm_in[:, 0:64])

            # Double-buffered flat padded images (+2 tail elems so the last
            # chunk's dy=2/dx=2 window stays in-bounds; zeroed once).
            xp = [[sb_pool.tile([128, FLAT + XTAIL], fp16,
                                name=f"xp{par}_{kt}", tag=f"xp{par}_{kt}")
                   for kt in range(KT)] for par in range(2)]
            for par in range(2):
                for kt in range(KT):
                    nc.vector.memset(xp[par][kt][:, FLAT:], 0.0)

            xap = x_d.ap()
            oap = o_d.ap()

            def band_dma(n, par, r0, nr):
                for kt in range(KT):
                    ring(kt).dma_start(
                        xp[par][kt][:, r0 * WP:(r0 + nr) * WP],
                        xap[n, kt * 128:(kt + 1) * 128,
                            r0 * WP:(r0 + nr) * WP])

            # Head ring order: image-0 band0 first (smallest first-MM
            # critical path), then mt0 weights with the first kernel taps
            # (pos 0-2) ahead so MM#1 waits for the fewest bytes, then the
            # rest.  All head DMAs race the PE warmup.
            band_dma(0, 0, *BANDS[0])
            for kt in range(KT):
                ring(kt).dma_start(w_sb[:, kt, 0, 0:3], w_d.ap()[:, kt, 0, 0:3])
            for kt in range(KT):
                ring(kt).dma_start(w_sb[:, kt, 0, 3:POS],
                                   w_d.ap()[:, kt, 0, 3:POS])
            for kt in range(KT):
                ring(kt).dma_start(w_sb[:, kt, 1], w_d.ap()[:, kt, 1])
            nc.scalar.dma_start(b_sb[:], b_d.ap())

            for n in range(N_PER_CORE):
                par = n % 2
                for r0, nr in (BANDS[1:] if n == 0 else BANDS):
                    band_dma(n, par, r0, nr)
                for ch in range(NCH):
                    y0 = ch * RB
                    for mt in range(MT):
                        ps = psum_pool.tile([128, RB, WP], fp32,
                                            name="ps", tag="ps")
                        first = True
                        for kt in range(KT):
                            for dy in range(K1):
                                for dx in range(K2):
                                    pos = dy * K2 + dx
                                    last = (kt == KT - 1 and pos == POS - 1)
                                    o = (y0 + dy) * WP + dx
                                    nc.tensor.matmul(
                                        ps[:, :, :],
                                        w_sb[:, kt, mt, pos, :],
                                        xp[par][kt][:, o:o + NWIN],
                                        start=first, stop=last,
                                    )
                                    first = False
                        ot = sb_pool.tile([128, RB, W], fp32,
                                          name="ot", tag="ot", bufs=4)
                        nc.vector.tensor_scalar_add(ot[:], ps[:, :, 0:W],
                                                    b_sb[:, mt:mt + 1])
                        dst = oap[n, mt * 128:(mt + 1) * 128, y0:y0 + RB, :]
                        if n == N_PER_CORE - 1 and ch == NCH - 1:
                            # Last chunk: split across idle rings to halve
                            # the final DMA drain tail.
                            half = RB // 2
                            eng0 = ring(mt)
                            eng1 = nc.gpsimd
                            eng0.dma_start(dst[:, 0:half, :],
                                           ot[:, 0:half, :])
                            eng1.dma_start(dst[:, half:RB, :],
                                           ot[:, half:RB, :])
                        else:
                            ring(mt).dma_start(dst, ot[:])
    nc.compile()
    return nc


def get_nc():
    if "nc" not in _NC_CACHE:
        _NC_CACHE["nc"] = _build_nc()
    return _NC_CACHE["nc"]


def kernel(x, weight, alphas, betas, bias):
    from concourse.bass_utils import run_bass_kernel_spmd

    xpad, w_arr, b_arr = _host_prep(x, weight, alphas, betas, bias)
    nc = get_nc()
    in_maps = [
        {"x": xpad[i * N_PER_CORE:(i + 1) * N_PER_CORE], "w": w_arr,
         "b": b_arr}
        for i in range(N_CORES)
    ]
    res = run_bass_kernel_spmd(nc, in_maps, core_ids=list(range(N_CORES)))
    LAST_RESULT["res"] = res
    return np.concatenate([r["out"] for r in res.results], axis=0)


# revision 6
# speedup vs baseline: 1.6247x; 1.4138x over previous
"""Trainium2 Bass kernel: 3x3 conv (NCHW 32x256x56x56, 256->256ch, pad 1) with
a host-expanded synthesized weight, data-parallel over 8 NeuronCores.

1D Winograd F(2,3) along x: host de-interleaves the zero-padded image into
even/odd column phases; the device computes the 4 Winograd input planes
V0..V3 with DVE adds (fp16), runs 4 point-GEMMs per output chunk (each
accumulating 2 ci-tiles x 3 dy taps in PSUM, N = 14 rows x 28 tiles = 392),
and reconstructs even/odd output columns with the inverse transform
  out_even = m0 + m1 + m2 + bias,   out_odd = m1 - m2 - m3 + bias
on DVE/GpSimd (bias fused via scalar_tensor_tensor).  This cuts PE matmul
columns 1.5x vs direct conv (2 column-streams per output column instead of
3).  The phase-split output layout is unpermuted on the host.

fp16 operands, fp32 accumulate; all matmul rhs windows are contiguous.
"""

import numpy as np

# Problem constants (hardcoded per contract; kernel.py must be self-contained)
OOC, OIC, K1, K2 = 64, 64, 3, 3
R0, R1 = 4, 4
N_CORES = 8
BATCH = 32
N_PER_CORE = BATCH // N_CORES  # 4
C = 256
H = W = 56
HP = H + 2        # 58 padded rows
TX = 28           # output x-tiles per row (F(2,3): 2 outputs/tile)
EO = 29           # even/odd phase columns (29 each)
FLAT = HP * 2 * EO  # 3364 fp16 elems per channel
RB = 14           # output rows per chunk -> N = RB*TX = 392
NCH = H // RB     # 4 chunks
KT = C // 128     # 2 input-channel tiles
MT = C // 128     # 2 output-channel tiles
NP = 4            # Winograd points
NWIN = RB * TX    # 392 matmul columns

# Input DMA row-bands: first covers chunk 0's rows (+dy halo) so compute
# starts early.
ROW_BANDS = [(0, 19), (19, 20), (39, 19)]

_NC_CACHE = {}
LAST_RESULT = {}  # test.py introspection: last BassKernelResults


def _expand_weight(weight, alphas, betas):
    """W[p0*64+i, p1*64+j, ky, kx] = w[i,j,ky,kx] * a[p0,p1] / (1+exp(w*b[p0,p1]))."""
    w = weight.astype(np.float32)[None, None]            # (1,1,64,64,3,3)
    a = alphas.astype(np.float32).reshape(R0, R1)[:, :, None, None, None, None]
    b = betas.astype(np.float32).reshape(R0, R1)[:, :, None, None, None, None]
    act = w * a / (1.0 + np.exp(w * b))                  # (4,4,64,64,3,3)
    return act.transpose(0, 2, 1, 3, 4, 5).reshape(R0 * OOC, R1 * OIC, K1, K2)


def _host_prep(x, weight, alphas, betas, bias):
    x = np.asarray(x, dtype=np.float32).astype(np.float16)
    xpad = np.pad(x, ((0, 0), (0, 0), (1, 1), (1, 1)))   # (B,C,58,58)
    # de-interleave columns into even/odd phases: (B,C,58,2,29)
    xeo = np.ascontiguousarray(
        xpad.reshape(BATCH, C, HP, EO, 2).transpose(0, 1, 2, 4, 3)
    ).reshape(BATCH, C, FLAT)
    Wfull = _expand_weight(np.asarray(weight), np.asarray(alphas),
                           np.asarray(betas)).astype(np.float32)
    # U[p,dy][ci,co]: Winograd-transformed weights (G w along x)
    w0, w1, w2 = Wfull[:, :, :, 0], Wfull[:, :, :, 1], Wfull[:, :, :, 2]
    U = np.stack([w0, (w0 + w1 + w2) / 2, (w0 - w1 + w2) / 2, w2],
                 axis=0)                                  # (p, co, ci, dy)
    U = U.transpose(2, 0, 3, 1)                           # (ci, p, dy, co)
    # lhsT layout: [ci_local(128), kt, mt, p, dy, co_local(128)]
    w_arr = np.ascontiguousarray(
        U.reshape(KT, 128, NP, K1, MT, 128).transpose(1, 0, 4, 2, 3, 5)
    ).astype(np.float16)
    b_arr = np.ascontiguousarray(
        np.asarray(bias, dtype=np.float32).reshape(MT, 128).T)
    return xeo, w_arr, b_arr


def _build_nc():
    import concourse.mybir as mybir
    import concourse.tile as tile
    from concourse import bacc

    fp32 = mybir.dt.float32
    fp16 = mybir.dt.float16
    add = mybir.AluOpType.add
    sub = mybir.AluOpType.subtract

    nc = bacc.Bacc("TRN2", target_bir_lowering=False, debug=False,
                   num_devices=N_CORES)

    x_d = nc.dram_tensor("x", [N_PER_CORE, C, FLAT], fp16,
                         kind="ExternalInput")
    w_d = nc.dram_tensor("w", [128, KT, MT, NP, K1, 128], fp16,
                         kind="ExternalInput")
    b_d = nc.dram_tensor("b", [128, MT], fp32, kind="ExternalInput")
    # x axis holds (phase, tx) pairs; host unpermutes to interleaved x.
    o_d = nc.dram_tensor("out", [N_PER_CORE, C, H, W], fp32,
                         kind="ExternalOutput")

    def ring(kt):
        return nc.sync if kt == 0 else nc.scalar

    with tile.TileContext(nc) as tc:
        with (
            tc.tile_pool(name="sb", bufs=1) as sb_pool,
            tc.tile_pool(name="ps", bufs=8, space="PSUM") as psum_pool,
        ):
            w_sb = sb_pool.tile([128, KT, MT, NP, K1, 128], fp16,
                                name="w_sb", tag="w_sb")
            b_sb = sb_pool.tile([128, MT], fp32, name="b_sb", tag="b_sb")

            warm_in = sb_pool.tile([128, 128], fp16, name="warm_in",
                                   tag="warm_in")
            nc.vector.memset(warm_in[:], 0.0)

            # Double-buffered phase-split images [rows, phase, tx] and the
            # 4 Winograd V planes per ci-tile.
            xeo = [[sb_pool.tile([128, HP, 2, EO], fp16,
                                 name=f"xeo{par}_{kt}", tag=f"xeo{par}_{kt}")
                    for kt in range(KT)] for par in range(2)]
            vpl = [[[sb_pool.tile([128, HP, TX], fp16,
                                  name=f"v{par}_{kt}_{p}",
                                  tag=f"v{par}_{kt}_{p}")
                     for p in range(NP)] for kt in range(KT)]
                   for par in range(2)]

            xap = x_d.ap()
            oap = o_d.ap()

            def band_dma(n, par, r0, nr):
                for kt in range(KT):
                    ring(kt).dma_start(
                        xeo[par][kt][:, r0:r0 + nr],
                        xap[n, kt * 128:(kt + 1) * 128,
                            r0 * 2 * EO:(r0 + nr) * 2 * EO])

            def forward(par, r0, nr):
                # V planes for padded rows r0..r0+nr (DVE, fp16)
                for kt in range(KT):
                    x_t = xeo[par][kt]
                    e0 = x_t[:, r0:r0 + nr, 0, 0:TX]
                    e1 = x_t[:, r0:r0 + nr, 0, 1:TX + 1]
                    o0 = x_t[:, r0:r0 + nr, 1, 0:TX]
                    o1 = x_t[:, r0:r0 + nr, 1, 1:TX + 1]
                    v = vpl[par][kt]
                    nc.vector.tensor_tensor(v[0][:, r0:r0 + nr], e0, e1, sub)
                    nc.vector.tensor_tensor(v[1][:, r0:r0 + nr], o0, e1, add)
                    nc.vector.tensor_tensor(v[2][:, r0:r0 + nr], e1, o0, sub)
                    nc.vector.tensor_tensor(v[3][:, r0:r0 + nr], o0, o1, sub)

            # PE warmup: junk matmuls bridge the HAM clock-gate window while
            # the first DMAs land.
            warm_ps = psum_pool.tile([128, RB, TX], fp32, name="warm_ps",
                                     tag="m")
            for _ in range(20):
                nc.tensor.matmul(warm_ps[:, 0, :], warm_in[:],
                                 warm_in[:, 0:TX])

            # Head: image-0 band0 + the first point's weights first, so the
            # first GEMM chain waits on the fewest bytes.
            band_dma(0, 0, *ROW_BANDS[0])
            for p in range(NP):
                for kt in range(KT):
                    ring(kt).dma_start(w_sb[:, kt, 0, p], w_d.ap()[:, kt, 0, p])
            nc.scalar.dma_start(b_sb[:], b_d.ap())
            forward(0, *ROW_BANDS[0])
            for r0, nr in ROW_BANDS[1:]:
                band_dma(0, 0, r0, nr)
                forward(0, r0, nr)
            for p in range(NP):
                for kt in range(KT):
                    ring(kt).dma_start(w_sb[:, kt, 1, p], w_d.ap()[:, kt, 1, p])

            for n in range(N_PER_CORE):
                par = n % 2
                if n > 0:
                    for r0, nr in ROW_BANDS:
                        band_dma(n, par, r0, nr)
                        forward(par, r0, nr)
                for ch in range(NCH):
                    y0 = ch * RB
                    for mt in range(MT):
                        m = []
                        for p in range(NP):
                            mp = psum_pool.tile([128, RB, TX], fp32,
                                                name="m", tag="m")
                            m.append(mp)
                            for kt in range(KT):
                                for dy in range(K1):
                                    nc.tensor.matmul(
                                        mp[:, :, :],
                                        w_sb[:, kt, mt, p, dy, :],
                                        vpl[par][kt][p][:, y0 + dy:
                                                        y0 + dy + RB, :],
                                        start=(kt == 0 and dy == 0),
                                        stop=(kt == KT - 1 and dy == K1 - 1),
                                    )
                        # Inverse transform, each op reading <=1 PSUM
                        # operand (HW limit), spread over ACT/DVE/GpSimd:
                        #   out_e = ((m0+bias) + m1) + m2
                        #   out_o = ((m1+bias) - m2) - m3
                        s1 = sb_pool.tile([128, RB, TX], fp32, name="s1",
                                          tag="s1", bufs=3)
                        s2 = sb_pool.tile([128, RB, TX], fp32, name="s2",
                                          tag="s2", bufs=3)
                        r1 = sb_pool.tile([128, RB, TX], fp32, name="r1",
                                          tag="r1", bufs=3)
                        r2 = sb_pool.tile([128, RB, TX], fp32, name="r2",
                                          tag="r2", bufs=3)
                        ot = sb_pool.tile([128, RB, 2, TX], fp32, name="ot",
                                          tag="ot", bufs=4)
                        bias_ap = b_sb[:, mt:mt + 1]
                        nc.scalar.add(s1[:], m[0][:], bias_ap)
                        nc.vector.tensor_tensor(s2[:], s1[:], m[1][:], add)
                        nc.vector.tensor_tensor(ot[:, :, 0, :], s2[:],
                                                m[2][:], add)
                        nc.scalar.add(r1[:], m[1][:], bias_ap)
                        nc.vector.tensor_tensor(r2[:], r1[:], m[2][:], sub)
                        nc.vector.tensor_tensor(ot[:, :, 1, :], r2[:],
                                                m[3][:], sub)
                        dst = oap[n, mt * 128:(mt + 1) * 128, y0:y0 + RB, :]
                        if n == N_PER_CORE - 1 and ch == NCH - 1:
                            half = RB // 2
                            ring(mt).dma_start(dst[:, 0:half, :],
                                               ot[:, 0:half])
                            nc.gpsimd.dma_start(dst[:, half:RB, :],
                                                ot[:, half:RB])
                        else:
                            ring(mt).dma_start(dst, ot[:])
    nc.compile()
    return nc


def get_nc():
    if "nc" not in _NC_CACHE:
        _NC_CACHE["nc"] = _build_nc()
    return _NC_CACHE["nc"]


def kernel(x, weight, alphas, betas, bias):
    from concourse.bass_utils import run_bass_kernel_spmd

    xeo, w_arr, b_arr = _host_prep(x, weight, alphas, betas, bias)
    nc = get_nc()
    in_maps = [
        {"x": xeo[i * N_PER_CORE:(i + 1) * N_PER_CORE], "w": w_arr,
         "b": b_arr}
        for i in range(N_CORES)
    ]
    res = run_bass_kernel_spmd(nc, in_maps, core_ids=list(range(N_CORES)))
    LAST_RESULT["res"] = res
    out = np.concatenate([r["out"] for r in res.results], axis=0)
    # device x-axis is (phase, tx) packed; interleave back to x = 2*tx+phase
    out = out.reshape(BATCH, C, H, 2, TX).transpose(0, 1, 2, 4, 3)
    return np.ascontiguousarray(out).reshape(BATCH, C, H, W)
